# revision 12
# baseline (speedup 1.0000x reference)
"""Channel-attention kernel for Trainium2 (8 NeuronCores).

Reference computation (per batch b):
    q = inputs[b].reshape(N, C)              # N = D*H*W = 4608, C = 64
    E = q @ q.T                              # (N, N)
    A = softmax(E, axis=-1)
    out[b] = gamma * (A @ q) + inputs[b]

Sharding: 8 cores = 4 batches x 2 row-halves of the attention matrix.
Each core computes softmax rows [r0, r0+2304) for one batch; softmax is
row-wise so cores are independent.

Per-core algorithm (single pass, transposed layout):
  * The QK^T matmul emits y = S_EXP*(E[n,m] - sq[n]) + B_EXP directly:
    operands are scaled by sqrt(S_EXP) on the host and an augmented
    contraction row adds B_EXP - S_EXP*sq[n] per column. sq[n] = |q_n|^2
    is the row stabilizer (the diagonal dominates each softmax row).
    y is exactly the Schraudolph integer form of exp(E - sq) for the U
    dtype: bits8 = y (fp8e4m3) or bits16 = y (bf16).
  * exp runs on THREE engines in parallel, one instruction per
    [128,2,512] PSUM tile:
      - ACT:    u = Exp(y*scale + bias)  (table-based, exact)
      - DVE:    u_bits = saturate_int(max(y, 0)), bitcast to fp8/bf16
      - GPSIMD: same as DVE
    The fast-exp linearization error (<6%) and the fp8 flush-to-zero
    below 2^-7 only touch off-diagonal softmax weights, all of which are
    < 6e-3 for this operator, so the final error stays ~1e-2 * gamma *
    that, far below tolerance.
  * PV matmul with lhsT = [q | 1] accumulates the unnormalized output
    (rows 0..63) and the softmax denominator S[n] (row 64) in one PSUM
    group. fp8 mode uses DoubleRow (two 128-chunks contracted per
    matmul).
  * Small PE transpose back to [n, c], then out = U/S * gamma + x.
"""

import sys

for _p in ("/opt/trn_rl_repo",):
    if _p not in sys.path:
        sys.path.insert(0, _p)

import numpy as np
import ml_dtypes
from contextlib import ExitStack

import concourse.bacc as bacc
import concourse.tile as tile
from concourse import mybir
from concourse import bass_utils

B, D, H, W_, C = 4, 8, 24, 24, 64
N = D * H * W_            # 4608
NCORES = 8
R = N // 2                # 2304 softmax rows per core
MCH = N // 128            # 36 contraction chunks
SUPERS = [512, 512, 512, 512, 256]   # n-column superblocks (sum = R)
DT = mybir.dt
AF = mybir.ActivationFunctionType
LN2 = float(np.log(2.0))

# Modes: QK matmul dtype ("fp8dr" = e4m3 DoubleRow, "bf16"), PV matmul
# dtype ("fp8dr", "bf16"). EXP_W = (ACT, DVE, GPSIMD) share of exp tiles.
QK_MODE = "bf16"
PV_MODE = "bf16"
# GPSIMD cannot read PSUM (BIR verifier), so it gets no exp tiles unless
# they are staged through SBUF first.
EXP_W = (0.56, 0.44, 0.0)

F8 = ml_dtypes.float8_e4m3
BF = ml_dtypes.bfloat16


def _exp_consts(pv_mode):
    if pv_mode == "fp8dr":
        return 8.0 / LN2, 56.0          # fp8e4m3: 3 mantissa bits, bias 7
    return 128.0 / LN2, 16256.0         # bf16: 7 mantissa bits, bias 127


def _build(qk_mode=None, pv_mode=None, exp_w=None):
    qk_mode = qk_mode or QK_MODE
    pv_mode = pv_mode or PV_MODE
    exp_w = exp_w or EXP_W
    s_exp, b_exp = _exp_consts(pv_mode)
    u_fp = DT.float8e4 if pv_mode == "fp8dr" else DT.bfloat16
    u_int = DT.int8 if pv_mode == "fp8dr" else DT.int16
    DR = mybir.MatmulPerfMode.DoubleRow

    nc = bacc.Bacc("TRN2", target_bir_lowering=False, debug=False)

    if qk_mode == "fp8dr":
        lhs = nc.dram_tensor("lhs", (33, 2, N), DT.float8e4, kind="ExternalInput").ap()
        rhs = nc.dram_tensor("rhs", (33, 2, R), DT.float8e4, kind="ExternalInput").ap()
    else:
        lhs = nc.dram_tensor("lhs", (65, N), DT.bfloat16, kind="ExternalInput").ap()
        rhs = nc.dram_tensor("rhs", (65, R), DT.bfloat16, kind="ExternalInput").ap()
    # DoubleRow ldweights requires a 128-wide stationary per slot, so the
    # fp8 PV lhsT is zero-padded from 65 to 128 columns (cols 65..127 make
    # harmless extra PSUM rows; streaming cost is set by the moving dims).
    qa_w = 128 if pv_mode == "fp8dr" else 65
    qa = nc.dram_tensor("qa", (128, MCH, qa_w), u_fp, kind="ExternalInput").ap()
    x_res = nc.dram_tensor("x_res", (R, C), DT.float32, kind="ExternalInput").ap()
    ident = nc.dram_tensor("ident", (128, 128), DT.bfloat16, kind="ExternalInput").ap()
    out = nc.dram_tensor("out", (R, C), DT.float32, kind="ExternalOutput").ap()

    # Deterministic weighted round-robin over exp tiles: 0=ACT 1=DVE 2=GPS
    n_tiles = sum((MCH * wd + 1023) // 1024 for wd in SUPERS)
    acc = [0.0, 0.0, 0.0]
    engine_of = []
    for _ in range(n_tiles):
        for k in range(3):
            acc[k] += exp_w[k]
        k = max(range(3), key=lambda j: acc[j])
        acc[k] -= 1.0
        engine_of.append(k)

    with ExitStack() as ctx:
        tc = ctx.enter_context(tile.TileContext(nc))
        singles = ctx.enter_context(tc.tile_pool(name="singles", bufs=1))
        upool = ctx.enter_context(tc.tile_pool(name="u", bufs=6))
        opool = ctx.enter_context(tc.tile_pool(name="o", bufs=4))
        spool = ctx.enter_context(tc.tile_pool(name="s", bufs=4))
        # PSUM: pe 3x2 banks + po 2x1 = 8. po is double-buffered so the
        # next super's PV accumulation never waits on the previous super's
        # epilogue; the epilogue's transposes are written into the just-
        # drained po buffer (cols 376+) instead of a dedicated bank.
        pe_pool = ctx.enter_context(tc.tile_pool(name="pe", bufs=3, space="PSUM"))
        po_pool = ctx.enter_context(tc.tile_pool(name="po", bufs=2, space="PSUM"))

        # Constants + PE warmup first so the PE ramps while DMAs issue.
        BIAS = singles.tile([128, 1], DT.float32)
        nc.vector.memset(BIAS, -b_exp / s_exp)
        sc0 = spool.tile([128, 1], DT.float32, tag="sc0")
        nc.vector.memset(sc0, 0.0)
        tl0 = spool.tile([128, 1], DT.float32, tag="tl0")
        nc.scalar.activation(tl0, sc0, AF.Exp, bias=BIAS[:, 0:1], scale=1.0 / s_exp)
        # Input DMAs: QK operands on the sync ring (smallest-first so the
        # main loop starts early); PV/epilogue operands on the gpsimd ring.
        if qk_mode == "fp8dr":
            LA = singles.tile([33, 2, N], DT.float8e4)
            RA = singles.tile([33, 2, R], DT.float8e4)
            nc.sync.dma_start(out=RA[:, :, :512], in_=rhs[:, :, :512])
            nc.sync.dma_start(out=LA[:, :, :512], in_=lhs[:, :, :512])
            nc.sync.dma_start(out=LA[:, :, 512:2560], in_=lhs[:, :, 512:2560])
            nc.sync.dma_start(out=LA[:, :, 2560:], in_=lhs[:, :, 2560:])
            nc.sync.dma_start(out=RA[:, :, 512:], in_=rhs[:, :, 512:])
        else:
            LA = singles.tile([65, N], DT.bfloat16)
            RA = singles.tile([65, R], DT.bfloat16)
            nc.sync.dma_start(out=RA[:, :512], in_=rhs[:, :512])
            nc.sync.dma_start(out=LA[:, :512], in_=lhs[:, :512])
            nc.sync.dma_start(out=LA[:, 512:2560], in_=lhs[:, 512:2560])
            nc.sync.dma_start(out=LA[:, 2560:], in_=lhs[:, 2560:])
            nc.sync.dma_start(out=RA[:, 512:], in_=rhs[:, 512:])

        QA = singles.tile([128, MCH, qa_w], u_fp)
        nc.gpsimd.dma_start(out=QA[:, 0:6, :], in_=qa[:, 0:6, :])
        nc.gpsimd.dma_start(out=QA[:, 6:, :], in_=qa[:, 6:, :])
        ID = singles.tile([128, 128], DT.bfloat16)
        nc.gpsimd.dma_start(out=ID, in_=ident)
        XR = singles.tile([128, R // 128, C], DT.float32)
        nc.gpsimd.dma_start(out=XR, in_=x_res.rearrange("(t p) c -> p t c", p=128))

        out_r = out.rearrange("(t p) c -> t p c", p=128)

        def epilogue(po_t, col_, Wd_, last):
            # po_t rows 0..63 = gamma * unnormalized out^T, row 64 = S[n]
            # (gamma is folded into qa on the host)
            ps = opool.tile([65, 512], DT.bfloat16, tag="ps", name="ps")
            nc.vector.tensor_copy(ps[:, :Wd_], po_t[:65, :Wd_])
            for j in range(Wd_ // 128):
                # transpose lands in the drained po buffer (f32 col 376+)
                tp = po_t[:, 376 + 34 * j : 409 + 34 * j].bitcast(DT.bfloat16)[:, :65]
                nc.tensor.transpose(tp, ps[:, j * 128 : (j + 1) * 128], ID[:65, :65])
                rs = spool.tile([128, 1], DT.float32, tag="rs", name="rs")
                nc.vector.reciprocal(rs, tp[:, 64:65])
                of = opool.tile([128, C], DT.float32, tag="of", name="of")
                blk = col_ // 128 + j
                if last:
                    # spread the tail chain across idle engines
                    nc.scalar.activation(of, tp[:, 0:64], AF.Copy, scale=rs[:, 0:1])
                    nc.gpsimd.tensor_add(of, of, XR[:, blk, :])
                    ring = nc.scalar if j % 2 else nc.sync
                else:
                    nc.vector.tensor_scalar_mul(of, tp[:, 0:64], rs)
                    nc.vector.tensor_add(of, of, XR[:, blk, :])
                    ring = nc.sync
                ring.dma_start(out=out_r[blk], in_=of)

        col = 0
        tile_idx = 0
        for s, Wd in enumerate(SUPERS):
            po_t = po_pool.tile([128, 512], DT.float32, tag="po")
            nsl = slice(col, col + Wd)
            per_bank = 512 // Wd
            n_pack = 2 * per_bank     # chunks per [128, 2, 512] PSUM tile
            for t in range(0, MCH, n_pack):
                e = pe_pool.tile([128, 2, 512], DT.float32, tag="pe")
                u = upool.tile([128, 2, 512], u_int, tag="u")
                for d_ in range(n_pack):
                    tc_ = t + d_
                    ch = slice(tc_ * 128, (tc_ + 1) * 128)
                    eo = e[:, d_ // per_bank, (d_ % per_bank) * Wd :][:, :Wd]
                    if qk_mode == "fp8dr":
                        nc.tensor.matmul(
                            eo, LA[:, :, ch], RA[:, :, nsl],
                            start=True, stop=True, perf_mode=DR,
                        )
                    else:
                        nc.tensor.matmul(
                            eo, LA[:, ch], RA[:, nsl], start=True, stop=True
                        )
                eng = engine_of[tile_idx]
                tile_idx += 1
                if eng == 0:
                    nc.scalar.activation(
                        u.bitcast(u_fp), e, AF.Exp,
                        bias=BIAS[:, 0:1], scale=1.0 / s_exp,
                    )
                elif eng == 1:
                    nc.vector.tensor_scalar_max(u, e, 0.0)
                else:
                    nc.gpsimd.tensor_scalar_max(u, e, 0.0)
                uf = u.bitcast(u_fp)
                if pv_mode == "fp8dr":
                    if per_bank == 1:
                        rhs_aps = [uf]
                    else:
                        rhs_aps = [
                            uf[:, k_, :].rearrange("p (s w) -> p s w", s=2)
                            for k_ in range(per_bank)
                        ]
                    for k_, rap in enumerate(rhs_aps):
                        tc_ = t + 2 * k_
                        nc.tensor.matmul(
                            po_t[:, :Wd],
                            QA[:, tc_ : tc_ + 2, :],
                            rap,
                            start=(tc_ == 0), stop=(tc_ == MCH - 2),
                            perf_mode=DR,
                        )
                else:
                    for d_ in range(n_pack):
                        tc_ = t + d_
                        nc.tensor.matmul(
                            po_t[:65, :Wd],
                            QA[:, tc_, :],
                            uf[:, d_ // per_bank, (d_ % per_bank) * Wd :][:, :Wd],
                            start=(tc_ == 0), stop=(tc_ == MCH - 1),
                        )
            epilogue(po_t, col, Wd, last=(s == len(SUPERS) - 1))
            col += Wd

    nc.compile()
    return nc


_CACHE = {}


def get_nc():
    key = (QK_MODE, PV_MODE, EXP_W)
    if key not in _CACHE:
        _CACHE[key] = _build()
    return _CACHE[key]


def make_in_maps(inputs_arr, gamma):
    s_exp, b_exp = _exp_consts(PV_MODE)
    a_scale = np.float32(np.sqrt(s_exp))
    u_np = F8 if PV_MODE == "fp8dr" else BF
    # aug values ~ (b_exp - s_exp*64); keep |value|/aug_v under e4m3's max
    # (240) when the QK operands are fp8. The fp8 quantization error here
    # is a per-column-uniform exponent shift that cancels in the softmax
    # normalization.
    aug_v = 2.0 if QK_MODE == "bf16" else (8.0 if PV_MODE == "fp8dr" else 32.0)

    q_all = np.ascontiguousarray(
        np.asarray(inputs_arr, dtype=np.float32).reshape(B, N, C)
    )
    ident = np.eye(128, dtype=BF)
    in_maps = []
    for core in range(NCORES):
        b, h = core // 2, core % 2
        qb = q_all[b]                               # (N, C)
        r0 = h * R
        m = dict(ident=ident, x_res=np.ascontiguousarray(qb[r0 : r0 + R]))

        if QK_MODE == "fp8dr":
            qs = (a_scale * qb).astype(F8)          # (N, 64) fp8, scaled
            qsT32 = qs.astype(np.float32).T         # (64, N)
            sqq = np.einsum("cn,cn->n", qsT32, qsT32).astype(np.float32)
            lhs_a = np.zeros((33, 2, N), np.float32)
            lhs_a[:32, 0] = qsT32[:32]
            lhs_a[:32, 1] = qsT32[32:]
            lhs_a[32, 0] = aug_v
            rhs_a = np.zeros((33, 2, R), np.float32)
            rhs_a[:32, 0] = qsT32[:32, r0 : r0 + R]
            rhs_a[:32, 1] = qsT32[32:, r0 : r0 + R]
            rhs_a[32, 0] = (b_exp - sqq[r0 : r0 + R]) / aug_v
            m["lhs"] = lhs_a.astype(F8)
            m["rhs"] = rhs_a.astype(F8)
        else:
            qs = (a_scale * qb).astype(BF)
            qsT32 = qs.astype(np.float32).T
            sqq = np.einsum("cn,cn->n", qsT32, qsT32).astype(np.float32)
            lhs_a = np.zeros((65, N), np.float32)
            lhs_a[:64] = qsT32
            lhs_a[64] = aug_v
            rhs_a = np.zeros((65, R), np.float32)
            rhs_a[:64] = qsT32[:, r0 : r0 + R]
            rhs_a[64] = (b_exp - sqq[r0 : r0 + R]) / aug_v
            m["lhs"] = lhs_a.astype(BF)
            m["rhs"] = rhs_a.astype(BF)

        qa_w = 128 if PV_MODE == "fp8dr" else 65
        qa8 = np.zeros((N, qa_w), np.float32)
        qa8[:, :64] = np.float32(gamma) * qb
        qa8[:, 64] = 1.0
        m["qa"] = np.ascontiguousarray(
            qa8.reshape(MCH, 128, qa_w).transpose(1, 0, 2)
        ).astype(u_np)
        in_maps.append(m)
    return in_maps


def run_hw(in_maps, **kwargs):
    nc = get_nc()
    return bass_utils.run_bass_kernel_spmd(
        nc, in_maps, core_ids=list(range(NCORES)), **kwargs
    )


def assemble(results):
    out_full = np.empty((B, N, C), np.float32)
    for core in range(NCORES):
        b, h = core // 2, core % 2
        out_full[b, h * R : (h + 1) * R] = results[core]["out"]
    return out_full.reshape(B, D, H, W_, C)


def kernel(**inputs):
    inputs_arr = np.asarray(inputs["inputs"], dtype=np.float32)
    gamma = np.asarray(inputs["gamma"], dtype=np.float32).reshape(-1)[0]
    in_maps = make_in_maps(inputs_arr, gamma)
    try:
        res = run_hw(in_maps)
    except Exception:
        import time

        time.sleep(5)
        res = run_hw(in_maps)
    return assemble(res.results)


# revision 13
# speedup vs baseline: 1.1759x; 1.1759x over previous
"""Channel-attention kernel for Trainium2 (8 NeuronCores).

Reference computation (per batch b):
    q = inputs[b].reshape(N, C)              # N = D*H*W = 4608, C = 64
    E = q @ q.T                              # (N, N)
    A = softmax(E, axis=-1)
    out[b] = gamma * (A @ q) + inputs[b]

Sharding: 8 cores = 4 batches x 2 row-halves of the attention matrix.
Each core computes softmax rows [r0, r0+2304) for one batch; softmax is
row-wise so cores are independent.

Per-core algorithm (single pass, transposed layout):
  * The QK^T matmul emits y = S_EXP*(E[n,m] - sq[n]) + B_EXP directly:
    operands are scaled by sqrt(S_EXP) on the host and an augmented
    contraction row adds B_EXP - S_EXP*sq[n] per column. sq[n] = |q_n|^2
    is the row stabilizer (the diagonal dominates each softmax row).
    y is exactly the Schraudolph integer form of exp(E - sq) for the U
    dtype: bits8 = y (fp8e4m3) or bits16 = y (bf16).
  * exp runs on THREE engines in parallel, one instruction per
    [128,2,512] PSUM tile:
      - ACT:    u = Exp(y*scale + bias)  (table-based, exact)
      - DVE:    u_bits = saturate_int(max(y, 0)), bitcast to fp8/bf16
      - GPSIMD: same as DVE
    The fast-exp linearization error (<6%) and the fp8 flush-to-zero
    below 2^-7 only touch off-diagonal softmax weights, all of which are
    < 6e-3 for this operator, so the final error stays ~1e-2 * gamma *
    that, far below tolerance.
  * PV matmul with lhsT = [q | 1] accumulates the unnormalized output
    (rows 0..63) and the softmax denominator S[n] (row 64) in one PSUM
    group. fp8 mode uses DoubleRow (two 128-chunks contracted per
    matmul).
  * Small PE transpose back to [n, c], then out = U/S * gamma + x.
"""

import sys

for _p in ("/opt/trn_rl_repo",):
    if _p not in sys.path:
        sys.path.insert(0, _p)

import numpy as np
import ml_dtypes
from contextlib import ExitStack

import concourse.bacc as bacc
import concourse.tile as tile
from concourse import mybir
from concourse import bass_utils

B, D, H, W_, C = 4, 8, 24, 24, 64
N = D * H * W_            # 4608
NCORES = 8
R = N // 2                # 2304 softmax rows per core
MCH = N // 128            # 36 contraction chunks
SUPERS = [512, 512, 512, 512, 256]   # n-column superblocks (sum = R)
DT = mybir.dt
AF = mybir.ActivationFunctionType
LN2 = float(np.log(2.0))

# Modes: QK matmul dtype ("fp8dr" = e4m3 DoubleRow, "bf16"), PV matmul
# dtype ("fp8dr", "bf16"). EXP_W = (ACT, DVE, GPSIMD) share of exp tiles.
QK_MODE = "bf16"
PV_MODE = "bf16"
# GPSIMD cannot read PSUM (BIR verifier), so it gets no exp tiles unless
# they are staged through SBUF first.
EXP_W = (0.56, 0.44, 0.0)

F8 = ml_dtypes.float8_e4m3
BF = ml_dtypes.bfloat16


def _exp_consts(pv_mode):
    if pv_mode == "fp8dr":
        return 8.0 / LN2, 56.0          # fp8e4m3: 3 mantissa bits, bias 7
    return 128.0 / LN2, 16256.0         # bf16: 7 mantissa bits, bias 127


def _build(qk_mode=None, pv_mode=None, exp_w=None):
    qk_mode = qk_mode or QK_MODE
    pv_mode = pv_mode or PV_MODE
    exp_w = exp_w or EXP_W
    s_exp, b_exp = _exp_consts(pv_mode)
    u_fp = DT.float8e4 if pv_mode == "fp8dr" else DT.bfloat16
    u_int = DT.int8 if pv_mode == "fp8dr" else DT.int16
    DR = mybir.MatmulPerfMode.DoubleRow

    nc = bacc.Bacc("TRN2", target_bir_lowering=False, debug=False)

    if qk_mode == "fp8dr":
        lhs = nc.dram_tensor("lhs", (33, 2, N), DT.float8e4, kind="ExternalInput").ap()
        rhs = nc.dram_tensor("rhs", (33, 2, R), DT.float8e4, kind="ExternalInput").ap()
    else:
        lhs = nc.dram_tensor("lhs", (65, N), DT.bfloat16, kind="ExternalInput").ap()
        rhs = nc.dram_tensor("rhs", (65, R), DT.bfloat16, kind="ExternalInput").ap()
    # DoubleRow ldweights requires a 128-wide stationary per slot, so the
    # fp8 PV lhsT is zero-padded from 65 to 128 columns (cols 65..127 make
    # harmless extra PSUM rows; streaming cost is set by the moving dims).
    qa_w = 128 if pv_mode == "fp8dr" else 65
    qa = nc.dram_tensor("qa", (128, MCH, qa_w), u_fp, kind="ExternalInput").ap()
    x_res = nc.dram_tensor("x_res", (R, C), DT.float32, kind="ExternalInput").ap()
    ident = nc.dram_tensor("ident", (128, 128), DT.bfloat16, kind="ExternalInput").ap()
    out = nc.dram_tensor("out", (R, C), DT.float32, kind="ExternalOutput").ap()

    # Deterministic weighted round-robin over exp tiles: 0=ACT 1=DVE 2=GPS
    n_tiles = sum((MCH * wd + 1023) // 1024 for wd in SUPERS)
    acc = [0.0, 0.0, 0.0]
    engine_of = []
    for _ in range(n_tiles):
        for k in range(3):
            acc[k] += exp_w[k]
        k = max(range(3), key=lambda j: acc[j])
        acc[k] -= 1.0
        engine_of.append(k)

    with ExitStack() as ctx:
        tc = ctx.enter_context(tile.TileContext(nc))
        singles = ctx.enter_context(tc.tile_pool(name="singles", bufs=1))
        upool = ctx.enter_context(tc.tile_pool(name="u", bufs=6))
        opool = ctx.enter_context(tc.tile_pool(name="o", bufs=4))
        spool = ctx.enter_context(tc.tile_pool(name="s", bufs=4))
        # PSUM: pe 3x2 banks + po 2x1 = 8. po is double-buffered so the
        # next super's PV accumulation never waits on the previous super's
        # epilogue; the epilogue's transposes are written into the just-
        # drained po buffer (cols 376+) instead of a dedicated bank.
        pe_pool = ctx.enter_context(tc.tile_pool(name="pe", bufs=3, space="PSUM"))
        po_pool = ctx.enter_context(tc.tile_pool(name="po", bufs=2, space="PSUM"))

        # Constants + PE warmup first so the PE ramps while DMAs issue.
        BIAS = singles.tile([128, 1], DT.float32)
        nc.vector.memset(BIAS, -b_exp / s_exp)
        sc0 = spool.tile([128, 1], DT.float32, tag="sc0")
        nc.vector.memset(sc0, 0.0)
        tl0 = spool.tile([128, 1], DT.float32, tag="tl0")
        nc.scalar.activation(tl0, sc0, AF.Exp, bias=BIAS[:, 0:1], scale=1.0 / s_exp)
        # Input DMAs: QK operands on the sync ring (smallest-first so the
        # main loop starts early); PV/epilogue operands on the gpsimd ring.
        if qk_mode == "fp8dr":
            LA = singles.tile([33, 2, N], DT.float8e4)
            RA = singles.tile([33, 2, R], DT.float8e4)
            nc.sync.dma_start(out=RA[:, :, :512], in_=rhs[:, :, :512])
            nc.sync.dma_start(out=LA[:, :, :512], in_=lhs[:, :, :512])
            nc.sync.dma_start(out=LA[:, :, 512:2560], in_=lhs[:, :, 512:2560])
            nc.sync.dma_start(out=LA[:, :, 2560:], in_=lhs[:, :, 2560:])
            nc.sync.dma_start(out=RA[:, :, 512:], in_=rhs[:, :, 512:])
        else:
            LA = singles.tile([65, N], DT.bfloat16)
            RA = singles.tile([65, R], DT.bfloat16)
            nc.sync.dma_start(out=RA[:, :512], in_=rhs[:, :512])
            nc.sync.dma_start(out=LA[:, :512], in_=lhs[:, :512])
            nc.sync.dma_start(out=LA[:, 512:2560], in_=lhs[:, 512:2560])
            nc.sync.dma_start(out=LA[:, 2560:], in_=lhs[:, 2560:])
            nc.sync.dma_start(out=RA[:, 512:], in_=rhs[:, 512:])

        QA = singles.tile([128, MCH, qa_w], u_fp)
        nc.gpsimd.dma_start(out=QA[:, 0:6, :], in_=qa[:, 0:6, :])
        nc.gpsimd.dma_start(out=QA[:, 6:, :], in_=qa[:, 6:, :])
        ID = singles.tile([128, 128], DT.bfloat16)
        nc.gpsimd.dma_start(out=ID, in_=ident)
        XR = singles.tile([128, R // 128, C], DT.float32)
        nc.gpsimd.dma_start(out=XR, in_=x_res.rearrange("(t p) c -> p t c", p=128))

        out_r = out.rearrange("(t p) c -> t p c", p=128)

        # HAM warmup: a short dummy bf16 matmul stream un-throttles the
        # core clock (dropping it measurably slows ALL engines ~20%).
        wz = singles.tile([128, 512], DT.bfloat16)
        nc.vector.memset(wz, 0.0)
        for w_ in range(5):
            ew = pe_pool.tile([128, 2, 512], DT.float32, tag="pe")
            for k_ in range(2):
                nc.tensor.matmul(ew[:, k_, :], wz[:, :128], wz, start=True, stop=True)

        def epilogue(po_t, col_, Wd_, last):
            # po_t rows 0..63 = gamma * unnormalized out^T, row 64 = S[n]
            # (gamma is folded into qa on the host)
            ps = opool.tile([65, 512], DT.bfloat16, tag="ps", name="ps")
            nc.vector.tensor_copy(ps[:, :Wd_], po_t[:65, :Wd_])
            for j in range(Wd_ // 128):
                # transpose lands in the drained po buffer (f32 col 376+)
                tp = po_t[:, 376 + 34 * j : 409 + 34 * j].bitcast(DT.bfloat16)[:, :65]
                nc.tensor.transpose(tp, ps[:, j * 128 : (j + 1) * 128], ID[:65, :65])
                rs = spool.tile([128, 1], DT.float32, tag="rs", name="rs")
                nc.vector.reciprocal(rs, tp[:, 64:65])
                of = opool.tile([128, C], DT.float32, tag="of", name="of")
                blk = col_ // 128 + j
                if last:
                    # spread the tail chain across idle engines
                    nc.scalar.activation(of, tp[:, 0:64], AF.Copy, scale=rs[:, 0:1])
                    nc.gpsimd.tensor_add(of, of, XR[:, blk, :])
                    ring = nc.scalar if j % 2 else nc.sync
                else:
                    nc.vector.tensor_scalar_mul(of, tp[:, 0:64], rs)
                    nc.vector.tensor_add(of, of, XR[:, blk, :])
                    ring = nc.sync
                ring.dma_start(out=out_r[blk], in_=of)

        col = 0
        tile_idx = 0
        for s, Wd in enumerate(SUPERS):
            po_t = po_pool.tile([128, 512], DT.float32, tag="po")
            nsl = slice(col, col + Wd)
            per_bank = 512 // Wd
            n_pack = 2 * per_bank     # chunks per [128, 2, 512] PSUM tile
            for t in range(0, MCH, n_pack):
                e = pe_pool.tile([128, 2, 512], DT.float32, tag="pe")
                u = upool.tile([128, 2, 512], u_int, tag="u")
                for d_ in range(n_pack):
                    tc_ = t + d_
                    ch = slice(tc_ * 128, (tc_ + 1) * 128)
                    eo = e[:, d_ // per_bank, (d_ % per_bank) * Wd :][:, :Wd]
                    if qk_mode == "fp8dr":
                        nc.tensor.matmul(
                            eo, LA[:, :, ch], RA[:, :, nsl],
                            start=True, stop=True, perf_mode=DR,
                        )
                    else:
                        nc.tensor.matmul(
                            eo, LA[:, ch], RA[:, nsl], start=True, stop=True
                        )
                eng = engine_of[tile_idx]
                tile_idx += 1
                if eng == 0:
                    nc.scalar.activation(
                        u.bitcast(u_fp), e, AF.Exp,
                        bias=BIAS[:, 0:1], scale=1.0 / s_exp,
                    )
                elif eng == 1:
                    nc.vector.tensor_scalar_max(u, e, 0.0)
                else:
                    nc.gpsimd.tensor_scalar_max(u, e, 0.0)
                uf = u.bitcast(u_fp)
                if pv_mode == "fp8dr":
                    if per_bank == 1:
                        rhs_aps = [uf]
                    else:
                        rhs_aps = [
                            uf[:, k_, :].rearrange("p (s w) -> p s w", s=2)
                            for k_ in range(per_bank)
                        ]
                    for k_, rap in enumerate(rhs_aps):
                        tc_ = t + 2 * k_
                        nc.tensor.matmul(
                            po_t[:, :Wd],
                            QA[:, tc_ : tc_ + 2, :],
                            rap,
                            start=(tc_ == 0), stop=(tc_ == MCH - 2),
                            perf_mode=DR,
                        )
                else:
                    for d_ in range(n_pack):
                        tc_ = t + d_
                        nc.tensor.matmul(
                            po_t[:65, :Wd],
                            QA[:, tc_, :],
                            uf[:, d_ // per_bank, (d_ % per_bank) * Wd :][:, :Wd],
                            start=(tc_ == 0), stop=(tc_ == MCH - 1),
                        )
            epilogue(po_t, col, Wd, last=(s == len(SUPERS) - 1))
            col += Wd

    nc.compile()
    return nc


_CACHE = {}


def get_nc():
    key = (QK_MODE, PV_MODE, EXP_W)
    if key not in _CACHE:
        _CACHE[key] = _build()
    return _CACHE[key]


def make_in_maps(inputs_arr, gamma):
    s_exp, b_exp = _exp_consts(PV_MODE)
    a_scale = np.float32(np.sqrt(s_exp))
    u_np = F8 if PV_MODE == "fp8dr" else BF
    # aug values ~ (b_exp - s_exp*64); keep |value|/aug_v under e4m3's max
    # (240) when the QK operands are fp8. The fp8 quantization error here
    # is a per-column-uniform exponent shift that cancels in the softmax
    # normalization.
    aug_v = 2.0 if QK_MODE == "bf16" else (8.0 if PV_MODE == "fp8dr" else 32.0)

    q_all = np.ascontiguousarray(
        np.asarray(inputs_arr, dtype=np.float32).reshape(B, N, C)
    )
    ident = np.eye(128, dtype=BF)
    in_maps = []
    for core in range(NCORES):
        b, h = core // 2, core % 2
        qb = q_all[b]                               # (N, C)
        r0 = h * R
        m = dict(ident=ident, x_res=np.ascontiguousarray(qb[r0 : r0 + R]))

        if QK_MODE == "fp8dr":
            qs = (a_scale * qb).astype(F8)          # (N, 64) fp8, scaled
            qsT32 = qs.astype(np.float32).T         # (64, N)
            sqq = np.einsum("cn,cn->n", qsT32, qsT32).astype(np.float32)
            lhs_a = np.zeros((33, 2, N), np.float32)
            lhs_a[:32, 0] = qsT32[:32]
            lhs_a[:32, 1] = qsT32[32:]
            lhs_a[32, 0] = aug_v
            rhs_a = np.zeros((33, 2, R), np.float32)
            rhs_a[:32, 0] = qsT32[:32, r0 : r0 + R]
            rhs_a[:32, 1] = qsT32[32:, r0 : r0 + R]
            rhs_a[32, 0] = (b_exp - sqq[r0 : r0 + R]) / aug_v
            m["lhs"] = lhs_a.astype(F8)
            m["rhs"] = rhs_a.astype(F8)
        else:
            qs = (a_scale * qb).astype(BF)
            qsT32 = qs.astype(np.float32).T
            sqq = np.einsum("cn,cn->n", qsT32, qsT32).astype(np.float32)
            lhs_a = np.zeros((65, N), np.float32)
            lhs_a[:64] = qsT32
            lhs_a[64] = aug_v
            rhs_a = np.zeros((65, R), np.float32)
            rhs_a[:64] = qsT32[:, r0 : r0 + R]
            rhs_a[64] = (b_exp - sqq[r0 : r0 + R]) / aug_v
            m["lhs"] = lhs_a.astype(BF)
            m["rhs"] = rhs_a.astype(BF)

        qa_w = 128 if PV_MODE == "fp8dr" else 65
        qa8 = np.zeros((N, qa_w), np.float32)
        qa8[:, :64] = np.float32(gamma) * qb
        qa8[:, 64] = 1.0
        m["qa"] = np.ascontiguousarray(
            qa8.reshape(MCH, 128, qa_w).transpose(1, 0, 2)
        ).astype(u_np)
        in_maps.append(m)
    return in_maps


def run_hw(in_maps, **kwargs):
    nc = get_nc()
    return bass_utils.run_bass_kernel_spmd(
        nc, in_maps, core_ids=list(range(NCORES)), **kwargs
    )


def assemble(results):
    out_full = np.empty((B, N, C), np.float32)
    for core in range(NCORES):
        b, h = core // 2, core % 2
        out_full[b, h * R : (h + 1) * R] = results[core]["out"]
    return out_full.reshape(B, D, H, W_, C)


def kernel(**inputs):
    inputs_arr = np.asarray(inputs["inputs"], dtype=np.float32)
    gamma = np.asarray(inputs["gamma"], dtype=np.float32).reshape(-1)[0]
    in_maps = make_in_maps(inputs_arr, gamma)
    try:
        res = run_hw(in_maps)
    except Exception:
        import time

        time.sleep(5)
        res = run_hw(in_maps)
    return assemble(res.results)


# revision 14
# speedup vs baseline: 1.2265x; 1.0430x over previous
"""Channel-attention kernel for Trainium2 (8 NeuronCores).

Reference computation (per batch b):
    q = inputs[b].reshape(N, C)              # N = D*H*W = 4608, C = 64
    E = q @ q.T                              # (N, N)
    A = softmax(E, axis=-1)
    out[b] = gamma * (A @ q) + inputs[b]

Sharding: 8 cores = 4 batches x 2 row-halves of the attention matrix.
Each core computes softmax rows [r0, r0+2304) for one batch; softmax is
row-wise so cores are independent.

Per-core algorithm (single pass, transposed layout):
  * The QK^T matmul emits y = S_EXP*(E[n,m] - sq[n]) + B_EXP directly:
    operands are scaled by sqrt(S_EXP) on the host and an augmented
    contraction row adds B_EXP - S_EXP*sq[n] per column. sq[n] = |q_n|^2
    is the row stabilizer (the diagonal dominates each softmax row).
    y is exactly the Schraudolph integer form of exp(E - sq) for the U
    dtype: bits8 = y (fp8e4m3) or bits16 = y (bf16).
  * exp runs on THREE engines in parallel, one instruction per
    [128,2,512] PSUM tile:
      - ACT:    u = Exp(y*scale + bias)  (table-based, exact)
      - DVE:    u_bits = saturate_int(max(y, 0)), bitcast to fp8/bf16
      - GPSIMD: same as DVE
    The fast-exp linearization error (<6%) and the fp8 flush-to-zero
    below 2^-7 only touch off-diagonal softmax weights, all of which are
    < 6e-3 for this operator, so the final error stays ~1e-2 * gamma *
    that, far below tolerance.
  * PV matmul with lhsT = [q | 1] accumulates the unnormalized output
    (rows 0..63) and the softmax denominator S[n] (row 64) in one PSUM
    group. fp8 mode uses DoubleRow (two 128-chunks contracted per
    matmul).
  * Small PE transpose back to [n, c], then out = U/S * gamma + x.
"""

import sys

for _p in ("/opt/trn_rl_repo",):
    if _p not in sys.path:
        sys.path.insert(0, _p)

import numpy as np
import ml_dtypes
from contextlib import ExitStack

import concourse.bacc as bacc
import concourse.tile as tile
from concourse import mybir
from concourse import bass_utils

B, D, H, W_, C = 4, 8, 24, 24, 64
N = D * H * W_            # 4608
NCORES = 8
R = N // 2                # 2304 softmax rows per core
MCH = N // 128            # 36 contraction chunks
SUPERS = [512, 512, 512, 512, 256]   # n-column superblocks (sum = R)
DT = mybir.dt
AF = mybir.ActivationFunctionType
LN2 = float(np.log(2.0))

# Modes: QK matmul dtype ("fp8dr" = e4m3 DoubleRow, "bf16"), PV matmul
# dtype ("fp8dr", "bf16"). EXP_W = (ACT, DVE, GPSIMD) share of exp tiles.
QK_MODE = "bf16"
PV_MODE = "bf16"
# GPSIMD cannot read PSUM (BIR verifier), so it gets no exp tiles unless
# they are staged through SBUF first.
EXP_W = (0.56, 0.44, 0.0)

F8 = ml_dtypes.float8_e4m3
BF = ml_dtypes.bfloat16


def _exp_consts(pv_mode):
    if pv_mode == "fp8dr":
        return 8.0 / LN2, 56.0          # fp8e4m3: 3 mantissa bits, bias 7
    return 128.0 / LN2, 16256.0         # bf16: 7 mantissa bits, bias 127


def _build(qk_mode=None, pv_mode=None, exp_w=None):
    qk_mode = qk_mode or QK_MODE
    pv_mode = pv_mode or PV_MODE
    exp_w = exp_w or EXP_W
    s_exp, b_exp = _exp_consts(pv_mode)
    u_fp = DT.float8e4 if pv_mode == "fp8dr" else DT.bfloat16
    u_int = DT.int8 if pv_mode == "fp8dr" else DT.int16
    DR = mybir.MatmulPerfMode.DoubleRow

    nc = bacc.Bacc("TRN2", target_bir_lowering=False, debug=False)

    if qk_mode == "fp8dr":
        lhs = nc.dram_tensor("lhs", (33, 2, N), DT.float8e4, kind="ExternalInput").ap()
        rhs = nc.dram_tensor("rhs", (33, 2, R), DT.float8e4, kind="ExternalInput").ap()
    else:
        lhs = nc.dram_tensor("lhs", (65, N), DT.bfloat16, kind="ExternalInput").ap()
        rhs = nc.dram_tensor("rhs", (65, R), DT.bfloat16, kind="ExternalInput").ap()
    # DoubleRow ldweights requires a 128-wide stationary per slot, so the
    # fp8 PV lhsT is zero-padded from 65 to 128 columns (cols 65..127 make
    # harmless extra PSUM rows; streaming cost is set by the moving dims).
    qa_w = 128 if pv_mode == "fp8dr" else 65
    qa = nc.dram_tensor("qa", (128, MCH, qa_w), u_fp, kind="ExternalInput").ap()
    x_res = nc.dram_tensor("x_res", (R, C), DT.float32, kind="ExternalInput").ap()
    ident = nc.dram_tensor("ident", (128, 128), DT.bfloat16, kind="ExternalInput").ap()
    out = nc.dram_tensor("out", (R, C), DT.float32, kind="ExternalOutput").ap()

    # Deterministic weighted round-robin over exp tiles: 0=ACT 1=DVE 2=GPS
    n_tiles = sum((MCH * wd + 1023) // 1024 for wd in SUPERS)
    acc = [0.0, 0.0, 0.0]
    engine_of = []
    for _ in range(n_tiles):
        for k in range(3):
            acc[k] += exp_w[k]
        k = max(range(3), key=lambda j: acc[j])
        acc[k] -= 1.0
        engine_of.append(k)

    with ExitStack() as ctx:
        tc = ctx.enter_context(tile.TileContext(nc))
        singles = ctx.enter_context(tc.tile_pool(name="singles", bufs=1))
        upool = ctx.enter_context(tc.tile_pool(name="u", bufs=6))
        opool = ctx.enter_context(tc.tile_pool(name="o", bufs=4))
        spool = ctx.enter_context(tc.tile_pool(name="s", bufs=4))
        # PSUM: pe 3x2 banks + po 2x1 = 8. po is double-buffered so the
        # next super's PV accumulation never waits on the previous super's
        # epilogue; the epilogue's transposes are written into the just-
        # drained po buffer (cols 376+) instead of a dedicated bank.
        pe_pool = ctx.enter_context(tc.tile_pool(name="pe", bufs=3, space="PSUM"))
        po_pool = ctx.enter_context(tc.tile_pool(name="po", bufs=2, space="PSUM"))

        # Constants + PE warmup first so the PE ramps while DMAs issue.
        BIAS = singles.tile([128, 1], DT.float32)
        nc.vector.memset(BIAS, -b_exp / s_exp)
        sc0 = spool.tile([128, 1], DT.float32, tag="sc0")
        nc.vector.memset(sc0, 0.0)
        tl0 = spool.tile([128, 1], DT.float32, tag="tl0")
        nc.scalar.activation(tl0, sc0, AF.Exp, bias=BIAS[:, 0:1], scale=1.0 / s_exp)
        # Input DMAs: QK operands on the sync ring (smallest-first so the
        # main loop starts early); PV/epilogue operands on the gpsimd ring.
        if qk_mode == "fp8dr":
            LA = singles.tile([33, 2, N], DT.float8e4)
            RA = singles.tile([33, 2, R], DT.float8e4)
            nc.sync.dma_start(out=RA[:, :, :512], in_=rhs[:, :, :512])
            nc.sync.dma_start(out=LA[:, :, :512], in_=lhs[:, :, :512])
            nc.sync.dma_start(out=LA[:, :, 512:2560], in_=lhs[:, :, 512:2560])
            nc.sync.dma_start(out=LA[:, :, 2560:], in_=lhs[:, :, 2560:])
            nc.sync.dma_start(out=RA[:, :, 512:], in_=rhs[:, :, 512:])
        else:
            LA = singles.tile([65, N], DT.bfloat16)
            RA = singles.tile([65, R], DT.bfloat16)
            nc.sync.dma_start(out=RA[:, :512], in_=rhs[:, :512])
            nc.sync.dma_start(out=LA[:, :512], in_=lhs[:, :512])
            nc.sync.dma_start(out=LA[:, 512:2560], in_=lhs[:, 512:2560])
            nc.sync.dma_start(out=LA[:, 2560:], in_=lhs[:, 2560:])
            nc.sync.dma_start(out=RA[:, 512:], in_=rhs[:, 512:])

        QA = singles.tile([128, MCH, qa_w], u_fp)
        nc.gpsimd.dma_start(out=QA[:, 0:6, :], in_=qa[:, 0:6, :])
        nc.gpsimd.dma_start(out=QA[:, 6:, :], in_=qa[:, 6:, :])
        ID = singles.tile([128, 128], DT.bfloat16)
        nc.gpsimd.dma_start(out=ID, in_=ident)
        XR = singles.tile([128, R // 128, C], DT.float32)
        nc.gpsimd.dma_start(out=XR, in_=x_res.rearrange("(t p) c -> p t c", p=128))

        out_r = out.rearrange("(t p) c -> t p c", p=128)

        # HAM warmup: a short dummy bf16 matmul stream un-throttles the
        # core clock (dropping it measurably slows ALL engines ~20%).
        wz = singles.tile([128, 512], DT.bfloat16)
        nc.vector.memset(wz, 0.0)
        for w_ in range(5):
            ew = pe_pool.tile([128, 2, 512], DT.float32, tag="pe")
            for k_ in range(2):
                nc.tensor.matmul(ew[:, k_, :], wz[:, :128], wz, start=True, stop=True)

        def epilogue(po_t, col_, Wd_, last):
            # po_t rows 0..63 = gamma * unnormalized out^T, row 64 = S[n]
            # (gamma is folded into qa on the host)
            ps = opool.tile([65, 512], DT.bfloat16, tag="ps", name="ps")
            nc.vector.tensor_copy(ps[:, :Wd_], po_t[:65, :Wd_])
            for j in range(Wd_ // 128):
                # transpose lands in the drained po buffer (f32 col 376+)
                tp = po_t[:, 376 + 34 * j : 409 + 34 * j].bitcast(DT.bfloat16)[:, :65]
                nc.tensor.transpose(tp, ps[:, j * 128 : (j + 1) * 128], ID[:65, :65])
                rs = spool.tile([128, 1], DT.float32, tag="rs", name="rs")
                nc.vector.reciprocal(rs, tp[:, 64:65])
                of = opool.tile([128, C], DT.float32, tag="of", name="of")
                blk = col_ // 128 + j
                if last and j % 2:
                    # run the two final blocks on disjoint engine chains so
                    # they finish in parallel
                    nc.scalar.activation(of, tp[:, 0:64], AF.Copy, scale=rs[:, 0:1])
                    nc.gpsimd.tensor_add(of, of, XR[:, blk, :])
                    ring = nc.scalar
                else:
                    nc.vector.tensor_scalar_mul(of, tp[:, 0:64], rs)
                    nc.vector.tensor_add(of, of, XR[:, blk, :])
                    ring = nc.sync
                ring.dma_start(out=out_r[blk], in_=of)

        col = 0
        tile_idx = 0
        for s, Wd in enumerate(SUPERS):
            po_t = po_pool.tile([128, 512], DT.float32, tag="po")
            nsl = slice(col, col + Wd)
            per_bank = 512 // Wd
            n_pack = 2 * per_bank     # chunks per [128, 2, 512] PSUM tile
            for t in range(0, MCH, n_pack):
                e = pe_pool.tile([128, 2, 512], DT.float32, tag="pe")
                u = upool.tile([128, 2, 512], u_int, tag="u")
                for d_ in range(n_pack):
                    tc_ = t + d_
                    ch = slice(tc_ * 128, (tc_ + 1) * 128)
                    eo = e[:, d_ // per_bank, (d_ % per_bank) * Wd :][:, :Wd]
                    if qk_mode == "fp8dr":
                        nc.tensor.matmul(
                            eo, LA[:, :, ch], RA[:, :, nsl],
                            start=True, stop=True, perf_mode=DR,
                        )
                    else:
                        nc.tensor.matmul(
                            eo, LA[:, ch], RA[:, nsl], start=True, stop=True
                        )
                eng = engine_of[tile_idx]
                tile_idx += 1
                if eng == 0:
                    nc.scalar.activation(
                        u.bitcast(u_fp), e, AF.Exp,
                        bias=BIAS[:, 0:1], scale=1.0 / s_exp,
                    )
                elif eng == 1:
                    nc.vector.tensor_scalar_max(u, e, 0.0)
                else:
                    nc.gpsimd.tensor_scalar_max(u, e, 0.0)
                uf = u.bitcast(u_fp)
                if pv_mode == "fp8dr":
                    if per_bank == 1:
                        rhs_aps = [uf]
                    else:
                        rhs_aps = [
                            uf[:, k_, :].rearrange("p (s w) -> p s w", s=2)
                            for k_ in range(per_bank)
                        ]
                    for k_, rap in enumerate(rhs_aps):
                        tc_ = t + 2 * k_
                        nc.tensor.matmul(
                            po_t[:, :Wd],
                            QA[:, tc_ : tc_ + 2, :],
                            rap,
                            start=(tc_ == 0), stop=(tc_ == MCH - 2),
                            perf_mode=DR,
                        )
                else:
                    for d_ in range(n_pack):
                        tc_ = t + d_
                        nc.tensor.matmul(
                            po_t[:65, :Wd],
                            QA[:, tc_, :],
                            uf[:, d_ // per_bank, (d_ % per_bank) * Wd :][:, :Wd],
                            start=(tc_ == 0), stop=(tc_ == MCH - 1),
                        )
            epilogue(po_t, col, Wd, last=(s == len(SUPERS) - 1))
            col += Wd

    nc.compile()
    return nc


_CACHE = {}


def get_nc():
    key = (QK_MODE, PV_MODE, EXP_W)
    if key not in _CACHE:
        _CACHE[key] = _build()
    return _CACHE[key]


def make_in_maps(inputs_arr, gamma):
    s_exp, b_exp = _exp_consts(PV_MODE)
    a_scale = np.float32(np.sqrt(s_exp))
    u_np = F8 if PV_MODE == "fp8dr" else BF
    # aug values ~ (b_exp - s_exp*64); keep |value|/aug_v under e4m3's max
    # (240) when the QK operands are fp8. The fp8 quantization error here
    # is a per-column-uniform exponent shift that cancels in the softmax
    # normalization.
    aug_v = 2.0 if QK_MODE == "bf16" else (8.0 if PV_MODE == "fp8dr" else 32.0)

    q_all = np.ascontiguousarray(
        np.asarray(inputs_arr, dtype=np.float32).reshape(B, N, C)
    )
    ident = np.eye(128, dtype=BF)
    in_maps = []
    for core in range(NCORES):
        b, h = core // 2, core % 2
        qb = q_all[b]                               # (N, C)
        r0 = h * R
        m = dict(ident=ident, x_res=np.ascontiguousarray(qb[r0 : r0 + R]))

        if QK_MODE == "fp8dr":
            qs = (a_scale * qb).astype(F8)          # (N, 64) fp8, scaled
            qsT32 = qs.astype(np.float32).T         # (64, N)
            sqq = np.einsum("cn,cn->n", qsT32, qsT32).astype(np.float32)
            lhs_a = np.zeros((33, 2, N), np.float32)
            lhs_a[:32, 0] = qsT32[:32]
            lhs_a[:32, 1] = qsT32[32:]
            lhs_a[32, 0] = aug_v
            rhs_a = np.zeros((33, 2, R), np.float32)
            rhs_a[:32, 0] = qsT32[:32, r0 : r0 + R]
            rhs_a[:32, 1] = qsT32[32:, r0 : r0 + R]
            rhs_a[32, 0] = (b_exp - sqq[r0 : r0 + R]) / aug_v
            m["lhs"] = lhs_a.astype(F8)
            m["rhs"] = rhs_a.astype(F8)
        else:
            qs = (a_scale * qb).astype(BF)
            qsT32 = qs.astype(np.float32).T
            sqq = np.einsum("cn,cn->n", qsT32, qsT32).astype(np.float32)
            lhs_a = np.zeros((65, N), np.float32)
            lhs_a[:64] = qsT32
            lhs_a[64] = aug_v
            rhs_a = np.zeros((65, R), np.float32)
            rhs_a[:64] = qsT32[:, r0 : r0 + R]
            rhs_a[64] = (b_exp - sqq[r0 : r0 + R]) / aug_v
            m["lhs"] = lhs_a.astype(BF)
            m["rhs"] = rhs_a.astype(BF)

        qa_w = 128 if PV_MODE == "fp8dr" else 65
        qa8 = np.zeros((N, qa_w), np.float32)
        qa8[:, :64] = np.float32(gamma) * qb
        qa8[:, 64] = 1.0
        m["qa"] = np.ascontiguousarray(
            qa8.reshape(MCH, 128, qa_w).transpose(1, 0, 2)
        ).astype(u_np)
        in_maps.append(m)
    return in_maps


def run_hw(in_maps, **kwargs):
    nc = get_nc()
    return bass_utils.run_bass_kernel_spmd(
        nc, in_maps, core_ids=list(range(NCORES)), **kwargs
    )


def assemble(results):
    out_full = np.empty((B, N, C), np.float32)
    for core in range(NCORES):
        b, h = core // 2, core % 2
        out_full[b, h * R : (h + 1) * R] = results[core]["out"]
    return out_full.reshape(B, D, H, W_, C)


def kernel(**inputs):
    inputs_arr = np.asarray(inputs["inputs"], dtype=np.float32)
    gamma = np.asarray(inputs["gamma"], dtype=np.float32).reshape(-1)[0]
    in_maps = make_in_maps(inputs_arr, gamma)
    try:
        res = run_hw(in_maps)
    except Exception:
        import time

        time.sleep(5)
        res = run_hw(in_maps)
    return assemble(res.results)


# revision 15
# speedup vs baseline: 1.2270x; 1.0004x over previous
"""Channel-attention kernel for Trainium2 (8 NeuronCores).

Reference computation (per batch b):
    q = inputs[b].reshape(N, C)              # N = D*H*W = 4608, C = 64
    E = q @ q.T                              # (N, N)
    A = softmax(E, axis=-1)
    out[b] = gamma * (A @ q) + inputs[b]

Sharding: 8 cores = 4 batches x 2 row-halves of the attention matrix.
Each core computes softmax rows [r0, r0+2304) for one batch; softmax is
row-wise so cores are independent.

Per-core algorithm (single pass, transposed layout):
  * The QK^T matmul emits y = S_EXP*(E[n,m] - sq[n]) + B_EXP directly:
    operands are scaled by sqrt(S_EXP) on the host and an augmented
    contraction row adds B_EXP - S_EXP*sq[n] per column. sq[n] = |q_n|^2
    is the row stabilizer (the diagonal dominates each softmax row).
    y is exactly the Schraudolph integer form of exp(E - sq) for the U
    dtype: bits8 = y (fp8e4m3) or bits16 = y (bf16).
  * exp runs on THREE engines in parallel, one instruction per
    [128,2,512] PSUM tile:
      - ACT:    u = Exp(y*scale + bias)  (table-based, exact)
      - DVE:    u_bits = saturate_int(max(y, 0)), bitcast to fp8/bf16
      - GPSIMD: same as DVE
    The fast-exp linearization error (<6%) and the fp8 flush-to-zero
    below 2^-7 only touch off-diagonal softmax weights, all of which are
    < 6e-3 for this operator, so the final error stays ~1e-2 * gamma *
    that, far below tolerance.
  * PV matmul with lhsT = [q | 1] accumulates the unnormalized output
    (rows 0..63) and the softmax denominator S[n] (row 64) in one PSUM
    group. fp8 mode uses DoubleRow (two 128-chunks contracted per
    matmul).
  * Small PE transpose back to [n, c], then out = U/S * gamma + x.
"""

import sys

for _p in ("/opt/trn_rl_repo",):
    if _p not in sys.path:
        sys.path.insert(0, _p)

import numpy as np
import ml_dtypes
from contextlib import ExitStack

import concourse.bacc as bacc
import concourse.tile as tile
from concourse import mybir
from concourse import bass_utils

B, D, H, W_, C = 4, 8, 24, 24, 64
N = D * H * W_            # 4608
NCORES = 8
R = N // 2                # 2304 softmax rows per core
MCH = N // 128            # 36 contraction chunks
SUPERS = [512, 512, 512, 512, 256]   # n-column superblocks (sum = R)
DT = mybir.dt
AF = mybir.ActivationFunctionType
LN2 = float(np.log(2.0))

# Modes: QK matmul dtype ("fp8dr" = e4m3 DoubleRow, "bf16"), PV matmul
# dtype ("fp8dr", "bf16"). EXP_W = (ACT, DVE, GPSIMD) share of exp tiles.
QK_MODE = "bf16"
PV_MODE = "bf16"
# GPSIMD cannot read PSUM (BIR verifier), so it gets no exp tiles unless
# they are staged through SBUF first.
EXP_W = (0.56, 0.44, 0.0)

F8 = ml_dtypes.float8_e4m3
BF = ml_dtypes.bfloat16


def _exp_consts(pv_mode):
    if pv_mode == "fp8dr":
        return 8.0 / LN2, 56.0          # fp8e4m3: 3 mantissa bits, bias 7
    return 128.0 / LN2, 16256.0         # bf16: 7 mantissa bits, bias 127


def _build(qk_mode=None, pv_mode=None, exp_w=None):
    qk_mode = qk_mode or QK_MODE
    pv_mode = pv_mode or PV_MODE
    exp_w = exp_w or EXP_W
    s_exp, b_exp = _exp_consts(pv_mode)
    u_fp = DT.float8e4 if pv_mode == "fp8dr" else DT.bfloat16
    u_int = DT.int8 if pv_mode == "fp8dr" else DT.int16
    DR = mybir.MatmulPerfMode.DoubleRow

    nc = bacc.Bacc("TRN2", target_bir_lowering=False, debug=False)

    if qk_mode == "fp8dr":
        lhs = nc.dram_tensor("lhs", (33, 2, N), DT.float8e4, kind="ExternalInput").ap()
        rhs = nc.dram_tensor("rhs", (33, 2, R), DT.float8e4, kind="ExternalInput").ap()
    else:
        lhs = nc.dram_tensor("lhs", (65, N), DT.bfloat16, kind="ExternalInput").ap()
        rhs = nc.dram_tensor("rhs", (65, R), DT.bfloat16, kind="ExternalInput").ap()
    # DoubleRow ldweights requires a 128-wide stationary per slot, so the
    # fp8 PV lhsT is zero-padded from 65 to 128 columns (cols 65..127 make
    # harmless extra PSUM rows; streaming cost is set by the moving dims).
    qa_w = 128 if pv_mode == "fp8dr" else 65
    qa = nc.dram_tensor("qa", (128, MCH, qa_w), u_fp, kind="ExternalInput").ap()
    x_res = nc.dram_tensor("x_res", (R, C), DT.float32, kind="ExternalInput").ap()
    ident = nc.dram_tensor("ident", (128, 128), DT.bfloat16, kind="ExternalInput").ap()
    out = nc.dram_tensor("out", (R, C), DT.float32, kind="ExternalOutput").ap()

    # Deterministic weighted round-robin over exp tiles: 0=ACT 1=DVE 2=GPS
    n_tiles = sum((MCH * wd + 1023) // 1024 for wd in SUPERS)
    acc = [0.0, 0.0, 0.0]
    engine_of = []
    for _ in range(n_tiles):
        for k in range(3):
            acc[k] += exp_w[k]
        k = max(range(3), key=lambda j: acc[j])
        acc[k] -= 1.0
        engine_of.append(k)

    with ExitStack() as ctx:
        tc = ctx.enter_context(tile.TileContext(nc))
        singles = ctx.enter_context(tc.tile_pool(name="singles", bufs=1))
        upool = ctx.enter_context(tc.tile_pool(name="u", bufs=6))
        opool = ctx.enter_context(tc.tile_pool(name="o", bufs=4))
        spool = ctx.enter_context(tc.tile_pool(name="s", bufs=4))
        # PSUM: pe 3x2 banks + po 2x1 = 8. po is double-buffered so the
        # next super's PV accumulation never waits on the previous super's
        # epilogue; the epilogue's transposes are written into the just-
        # drained po buffer (cols 376+) instead of a dedicated bank.
        pe_pool = ctx.enter_context(tc.tile_pool(name="pe", bufs=3, space="PSUM"))
        po_pool = ctx.enter_context(tc.tile_pool(name="po", bufs=2, space="PSUM"))

        # Constants + PE warmup first so the PE ramps while DMAs issue.
        BIAS = singles.tile([128, 1], DT.float32)
        nc.vector.memset(BIAS, -b_exp / s_exp)
        sc0 = spool.tile([128, 1], DT.float32, tag="sc0")
        nc.vector.memset(sc0, 0.0)
        tl0 = spool.tile([128, 1], DT.float32, tag="tl0")
        nc.scalar.activation(tl0, sc0, AF.Exp, bias=BIAS[:, 0:1], scale=1.0 / s_exp)
        # Input DMAs: QK operands on the sync ring (smallest-first so the
        # main loop starts early); PV/epilogue operands on the gpsimd ring.
        if qk_mode == "fp8dr":
            LA = singles.tile([33, 2, N], DT.float8e4)
            RA = singles.tile([33, 2, R], DT.float8e4)
            nc.sync.dma_start(out=RA[:, :, :512], in_=rhs[:, :, :512])
            nc.sync.dma_start(out=LA[:, :, :512], in_=lhs[:, :, :512])
            nc.sync.dma_start(out=LA[:, :, 512:2560], in_=lhs[:, :, 512:2560])
            nc.sync.dma_start(out=LA[:, :, 2560:], in_=lhs[:, :, 2560:])
            nc.sync.dma_start(out=RA[:, :, 512:], in_=rhs[:, :, 512:])
        else:
            LA = singles.tile([65, N], DT.bfloat16)
            RA = singles.tile([65, R], DT.bfloat16)
            nc.sync.dma_start(out=RA[:, :512], in_=rhs[:, :512])
            nc.scalar.dma_start(out=LA[:, :512], in_=lhs[:, :512])
            nc.sync.dma_start(out=LA[:, 512:2560], in_=lhs[:, 512:2560])
            nc.sync.dma_start(out=LA[:, 2560:], in_=lhs[:, 2560:])
            nc.sync.dma_start(out=RA[:, 512:], in_=rhs[:, 512:])

        QA = singles.tile([128, MCH, qa_w], u_fp)
        nc.gpsimd.dma_start(out=QA[:, 0:6, :], in_=qa[:, 0:6, :])
        nc.gpsimd.dma_start(out=QA[:, 6:, :], in_=qa[:, 6:, :])
        ID = singles.tile([128, 128], DT.bfloat16)
        nc.gpsimd.dma_start(out=ID, in_=ident)
        XR = singles.tile([128, R // 128, C], DT.float32)
        nc.gpsimd.dma_start(out=XR, in_=x_res.rearrange("(t p) c -> p t c", p=128))

        out_r = out.rearrange("(t p) c -> t p c", p=128)

        # HAM warmup: a short dummy bf16 matmul stream un-throttles the
        # core clock (dropping it measurably slows ALL engines ~20%).
        wz = singles.tile([128, 512], DT.bfloat16)
        nc.vector.memset(wz, 0.0)
        for w_ in range(5):
            ew = pe_pool.tile([128, 2, 512], DT.float32, tag="pe")
            for k_ in range(2):
                nc.tensor.matmul(ew[:, k_, :], wz[:, :128], wz, start=True, stop=True)

        def epilogue(po_t, col_, Wd_, last):
            # po_t rows 0..63 = gamma * unnormalized out^T, row 64 = S[n]
            # (gamma is folded into qa on the host)
            ps = opool.tile([65, 512], DT.bfloat16, tag="ps", name="ps")
            nc.vector.tensor_copy(ps[:, :Wd_], po_t[:65, :Wd_])
            for j in range(Wd_ // 128):
                # transpose lands in the drained po buffer (f32 col 376+)
                tp = po_t[:, 376 + 34 * j : 409 + 34 * j].bitcast(DT.bfloat16)[:, :65]
                nc.tensor.transpose(tp, ps[:, j * 128 : (j + 1) * 128], ID[:65, :65])
                rs = spool.tile([128, 1], DT.float32, tag="rs", name="rs")
                nc.vector.reciprocal(rs, tp[:, 64:65])
                of = opool.tile([128, C], DT.float32, tag="of", name="of")
                blk = col_ // 128 + j
                if last and j % 2:
                    # run the two final blocks on disjoint engine chains so
                    # they finish in parallel
                    nc.scalar.activation(of, tp[:, 0:64], AF.Copy, scale=rs[:, 0:1])
                    nc.gpsimd.tensor_add(of, of, XR[:, blk, :])
                    ring = nc.scalar
                else:
                    nc.vector.tensor_scalar_mul(of, tp[:, 0:64], rs)
                    nc.vector.tensor_add(of, of, XR[:, blk, :])
                    ring = nc.sync
                ring.dma_start(out=out_r[blk], in_=of)

        col = 0
        tile_idx = 0
        for s, Wd in enumerate(SUPERS):
            po_t = po_pool.tile([128, 512], DT.float32, tag="po")
            nsl = slice(col, col + Wd)
            per_bank = 512 // Wd
            n_pack = 2 * per_bank     # chunks per [128, 2, 512] PSUM tile
            for t in range(0, MCH, n_pack):
                e = pe_pool.tile([128, 2, 512], DT.float32, tag="pe")
                u = upool.tile([128, 2, 512], u_int, tag="u")
                for d_ in range(n_pack):
                    tc_ = t + d_
                    ch = slice(tc_ * 128, (tc_ + 1) * 128)
                    eo = e[:, d_ // per_bank, (d_ % per_bank) * Wd :][:, :Wd]
                    if qk_mode == "fp8dr":
                        nc.tensor.matmul(
                            eo, LA[:, :, ch], RA[:, :, nsl],
                            start=True, stop=True, perf_mode=DR,
                        )
                    else:
                        nc.tensor.matmul(
                            eo, LA[:, ch], RA[:, nsl], start=True, stop=True
                        )
                eng = engine_of[tile_idx]
                tile_idx += 1
                if eng == 0:
                    nc.scalar.activation(
                        u.bitcast(u_fp), e, AF.Exp,
                        bias=BIAS[:, 0:1], scale=1.0 / s_exp,
                    )
                elif eng == 1:
                    nc.vector.tensor_scalar_max(u, e, 0.0)
                else:
                    nc.gpsimd.tensor_scalar_max(u, e, 0.0)
                uf = u.bitcast(u_fp)
                if pv_mode == "fp8dr":
                    if per_bank == 1:
                        rhs_aps = [uf]
                    else:
                        rhs_aps = [
                            uf[:, k_, :].rearrange("p (s w) -> p s w", s=2)
                            for k_ in range(per_bank)
                        ]
                    for k_, rap in enumerate(rhs_aps):
                        tc_ = t + 2 * k_
                        nc.tensor.matmul(
                            po_t[:, :Wd],
                            QA[:, tc_ : tc_ + 2, :],
                            rap,
                            start=(tc_ == 0), stop=(tc_ == MCH - 2),
                            perf_mode=DR,
                        )
                else:
                    for d_ in range(n_pack):
                        tc_ = t + d_
                        nc.tensor.matmul(
                            po_t[:65, :Wd],
                            QA[:, tc_, :],
                            uf[:, d_ // per_bank, (d_ % per_bank) * Wd :][:, :Wd],
                            start=(tc_ == 0), stop=(tc_ == MCH - 1),
                        )
            epilogue(po_t, col, Wd, last=(s == len(SUPERS) - 1))
            col += Wd

    nc.compile()
    return nc


_CACHE = {}


def get_nc():
    key = (QK_MODE, PV_MODE, EXP_W)
    if key not in _CACHE:
        _CACHE[key] = _build()
    return _CACHE[key]


def make_in_maps(inputs_arr, gamma):
    s_exp, b_exp = _exp_consts(PV_MODE)
    a_scale = np.float32(np.sqrt(s_exp))
    u_np = F8 if PV_MODE == "fp8dr" else BF
    # aug values ~ (b_exp - s_exp*64); keep |value|/aug_v under e4m3's max
    # (240) when the QK operands are fp8. The fp8 quantization error here
    # is a per-column-uniform exponent shift that cancels in the softmax
    # normalization.
    aug_v = 2.0 if QK_MODE == "bf16" else (8.0 if PV_MODE == "fp8dr" else 32.0)

    q_all = np.ascontiguousarray(
        np.asarray(inputs_arr, dtype=np.float32).reshape(B, N, C)
    )
    ident = np.eye(128, dtype=BF)
    in_maps = []
    for core in range(NCORES):
        b, h = core // 2, core % 2
        qb = q_all[b]                               # (N, C)
        r0 = h * R
        m = dict(ident=ident, x_res=np.ascontiguousarray(qb[r0 : r0 + R]))

        if QK_MODE == "fp8dr":
            qs = (a_scale * qb).astype(F8)          # (N, 64) fp8, scaled
            qsT32 = qs.astype(np.float32).T         # (64, N)
            sqq = np.einsum("cn,cn->n", qsT32, qsT32).astype(np.float32)
            lhs_a = np.zeros((33, 2, N), np.float32)
            lhs_a[:32, 0] = qsT32[:32]
            lhs_a[:32, 1] = qsT32[32:]
            lhs_a[32, 0] = aug_v
            rhs_a = np.zeros((33, 2, R), np.float32)
            rhs_a[:32, 0] = qsT32[:32, r0 : r0 + R]
            rhs_a[:32, 1] = qsT32[32:, r0 : r0 + R]
            rhs_a[32, 0] = (b_exp - sqq[r0 : r0 + R]) / aug_v
            m["lhs"] = lhs_a.astype(F8)
            m["rhs"] = rhs_a.astype(F8)
        else:
            qs = (a_scale * qb).astype(BF)
            qsT32 = qs.astype(np.float32).T
            sqq = np.einsum("cn,cn->n", qsT32, qsT32).astype(np.float32)
            lhs_a = np.zeros((65, N), np.float32)
            lhs_a[:64] = qsT32
            lhs_a[64] = aug_v
            rhs_a = np.zeros((65, R), np.float32)
            rhs_a[:64] = qsT32[:, r0 : r0 + R]
            rhs_a[64] = (b_exp - sqq[r0 : r0 + R]) / aug_v
            m["lhs"] = lhs_a.astype(BF)
            m["rhs"] = rhs_a.astype(BF)

        qa_w = 128 if PV_MODE == "fp8dr" else 65
        qa8 = np.zeros((N, qa_w), np.float32)
        qa8[:, :64] = np.float32(gamma) * qb
        qa8[:, 64] = 1.0
        m["qa"] = np.ascontiguousarray(
            qa8.reshape(MCH, 128, qa_w).transpose(1, 0, 2)
        ).astype(u_np)
        in_maps.append(m)
    return in_maps


def run_hw(in_maps, **kwargs):
    nc = get_nc()
    return bass_utils.run_bass_kernel_spmd(
        nc, in_maps, core_ids=list(range(NCORES)), **kwargs
    )


def assemble(results):
    out_full = np.empty((B, N, C), np.float32)
    for core in range(NCORES):
        b, h = core // 2, core % 2
        out_full[b, h * R : (h + 1) * R] = results[core]["out"]
    return out_full.reshape(B, D, H, W_, C)


def kernel(**inputs):
    inputs_arr = np.asarray(inputs["inputs"], dtype=np.float32)
    gamma = np.asarray(inputs["gamma"], dtype=np.float32).reshape(-1)[0]
    in_maps = make_in_maps(inputs_arr, gamma)
    try:
        res = run_hw(in_maps)
    except Exception:
        import time

        time.sleep(5)
        res = run_hw(in_maps)
    return assemble(res.results)
